# revision 48
# baseline (speedup 1.0000x reference)
"""Trainium2 Bass kernel for grouped-query causal self-attention.

Problem shapes (hardcoded): x [8,1024,1024] f32, W_attn [6144,1024] f32,
W_proj [1024,4096] f32. 16 heads, head_dim 64, 4 query sets sharing one K/V.

Sharding: data parallel over batch — one batch element per NeuronCore (8 cores).
No collectives needed.

Per-core algorithm (everything "transposed" = [feature, token] layout so no
on-device transposes are needed; x is pre-transposed on the host):
  1. qkvT tiles = W_attn @ x^T   (stationary = W_attn^T tile, moving = x^T)
     -> K^T [1024f, 1024t], Q_g^T per set, and V in normal [t, f] layout.
  2. Attention per (set g, head PAIR j=(2j,2j+1)), computed transposed:
     the two heads of a pair live in partitions 0-63 / 64-127 of the same
     kt/qt tile, so their 64-contraction QK^T matmuls run CONCURRENTLY on
     the PE via row tiling (tile_position (0,0) and (64,0)) — 2x QK rate.
        S^T[k, q] = K_tile^T-stationary @ Q^T-moving   (contraction = 64)
        causal: additive 0/-1e30 mask matmul (cmask@ident) accumulated onto
        the 128x128 diagonal blocks, on the PE so deps stay single-engine
        P^T = exp(S^T * scale)   one ACT op per k-tile covers both heads
        y_aug^T[d, q] = V_aug-stationary @ P^T-moving  (V augmented with a
        ones column -> row 64 of y_aug^T = softmax denominator for free;
        stationary slices padded to 128 cols for fast weight load)
        normalize: denominator row spread to [64,8] lanes via SBUF->SBUF DMA,
        bf16 reciprocal, DRAM hop for partition-broadcast, multiply (bf16).
  3. out = combined @ W_proj^T accumulated over sets in bf16 (osb), moving =
     W_proj^T streamed from DRAM in batched [128,8,512] loads.
dtypes: bf16 operands for matmuls (fp32 PSUM accumulate), fp32 softmax
denominator path, bf16 output accumulation (host converts to f32).

Scheduling notes (measured on HW, ~30us better than the naive ordering):
  - out-proj(g) is emitted inside attention(g+1)'s head-pair loop (cc=0 at
    j==1, cc=1 at j==5, or j==6 in the last window to bridge the gap into
    the final projection) and q-proj(3)'s second half at j==2 of
    attention(3), so the list scheduler holds PE filler back for the
    late-window stalls instead of draining it all at window start. The wp
    weight halves are dispatched one head-pair ahead of their chains, from
    the idle gpsimd queue so the 2.9us transfers never queue in front of
    the latency-critical normalize DMAs on sync.
  - set 3's W_proj tiles are prefetched through the wa pool from the idle
    gpsimd DMA queue (a pool-slot wait there blocks nothing), so the final
    projection never waits on DMA; it alternates over all four PSUM pools
    (op/ypl/qp/ypl, all idle by then) and streams each osb half to DRAM
    inline.
  - the softmax normalize is phased: both PSUM-releasing casts are emitted
    before any DMA-bounce-dependent op, so the ypl banks recycle without
    waiting on a DRAM round trip (the release gates the next block's PV).
  - x^T is loaded as ONE batched [128,8,1024] DMA (one dispatch, not 8);
    the startup weight loads go K-fg0, Q0-fg0, V, then K-fg1/Q0-fg1 (the
    fg1 halves aren't needed until j==4, while V gates the first PV), with
    wa bufs=4 so Q0-fg1's ring slot isn't held by a V load.
  - the two per-block reciprocal broadcasts are ONE [64,2,512] DMA; both
    den scatters land in one [64,16] tile feeding ONE reciprocal (their
    latencies overlap), and the normalize multiplies run on gpsimd
    (all-SBUF operands, ~10us slack) so no DMA-waiting op ever sits ahead
    of the next block's PSUM-releasing casts in the vector queue.
  - the wa/x weight DMAs stay on the sync queue: dispatching them from busy
    engine queues (scalar) head-of-line-blocks the exp ACTs behind them.
"""

import math

import ml_dtypes
import numpy as np

import concourse.bacc as bacc
import concourse.bass as bass
import concourse.mybir as mybir
import concourse.tile as tile
from concourse.bass_utils import run_bass_kernel_spmd

BF16 = ml_dtypes.bfloat16

B, T, C = 8, 1024, 1024
NH, HD, NQS = 16, 64, 4
SCALE = 1.0 / math.sqrt(HD)
NT = T // 128  # token tiles
NCH = C // 128  # channel tiles
KOFF = NQS * C  # 4096: K rows in W_attn
VOFF = (NQS + 1) * C  # 5120: V rows in W_attn

_CACHE = {}
LAST = {}  # exec_time_ns etc for test harness


def _build():
    f32 = mybir.dt.float32
    bf16 = mybir.dt.bfloat16
    EXP = mybir.ActivationFunctionType.Exp

    nc = bacc.Bacc()
    xT = nc.declare_dram_parameter("xT", [C, T], bf16, isOutput=False)
    waT = nc.declare_dram_parameter("waT", [C, 6 * C], bf16, isOutput=False)
    wpT = nc.declare_dram_parameter("wpT", [NQS * C, C], bf16, isOutput=False)
    identD = nc.declare_dram_parameter("ident", [128, 128], bf16, isOutput=False)
    cmaskD = nc.declare_dram_parameter("cmaskT", [128, 128], bf16, isOutput=False)
    out = nc.declare_dram_parameter("out", [T, C], bf16, isOutput=True)
    # DRAM bounce rows for the partition-broadcast of softmax reciprocals.
    rscr = nc.dram_tensor("rscr", [128, 512], bf16)

    with tile.TileContext(nc) as tc:
        with (
            tc.tile_pool(name="res", bufs=1) as res,
            tc.tile_pool(name="wa", bufs=4) as wa_pool,
            tc.tile_pool(name="wp", bufs=2) as wp_pool,
            tc.tile_pool(name="pt", bufs=6) as pt_pool,
            tc.tile_pool(name="yab", bufs=8) as yab_pool,
            tc.tile_pool(name="small", bufs=4) as small_pool,
            tc.tile_pool(name="qp", bufs=1, space="PSUM") as qp_pool,
            tc.tile_pool(name="op", bufs=1, space="PSUM") as op_pool,
            tc.tile_pool(name="ypl", bufs=2, space="PSUM") as ypl,
            tc.tile_pool(name="ptmp", bufs=2, space="PSUM") as ptmp,
        ):
            xtb = res.tile([128, NCH, T], bf16, tag="xtb", name="xtb")
            xt = [xtb[:, i, :] for i in range(NCH)]
            kt = [res.tile([128, T], bf16, tag=f"kt{i}", name=f"kt{i}") for i in range(NCH)]
            # [NH, 65] V_aug per head + 64 pad cols so every head h has a full
            # 128-wide stationary slice at offset h*65 (FWL-eligible LDW)
            vt = [res.tile([128, NH * (HD + 1) + 64], bf16, tag=f"vt{i}", name=f"vt{i}") for i in range(NT)]
            # double-buffered across query-set parity so q-proj(g+1) overlaps
            # attention(g), and out-proj(g) overlaps attention(g+1)
            qts = [
                [res.tile([128, T], bf16, tag=f"qt{p}_{i}", name=f"qt{p}_{i}") for i in range(NCH)]
                for p in range(2)
            ]
            yts = [
                [res.tile([128, T], bf16, tag=f"yt{p}_{i}", name=f"yt{p}_{i}") for i in range(NCH)]
                for p in range(2)
            ]
            osb = [res.tile([128, C], bf16, tag=f"osb{i}", name=f"osb{i}") for i in range(NT)]

            ident = res.tile([128, 128], bf16, tag="ident", name="ident")
            cmask = res.tile([128, 128], bf16, tag="cmask", name="cmask")
            vtv = [
                vt[tt][:, 0 : NH * (HD + 1)].rearrange("p (a b) -> p a b", b=HD + 1)
                for tt in range(NT)
            ]
            # x tiles first: they gate the first projection matmuls (the Sync
            # DMA queue is serial, so emission order = critical-path order)
            nc.sync.dma_start(out=xtb, in_=xT.rearrange("(a p) t -> p a t", p=128))
            # ones column + pad via on-chip memset instead of strided DMAs
            # (the DMA version cost ~1.4us apiece at the head of the queue)
            for tt in range(NT):
                nc.gpsimd.memset(vt[tt][:, NH * (HD + 1) :], 0.0)
                nc.gpsimd.memset(vtv[tt][:, :, HD : HD + 1], 1.0)

            def project_T(dst, fbase, tag, pool, ptag, fgs=(0, 1)):
                """dst[i][f_local, t] = (x @ W_attn.T).T rows fbase..fbase+1024."""
                for fg in fgs:  # 512-wide feature groups
                    f0 = fbase + fg * 512
                    w = wa_pool.tile([128, NCH, 512], bf16, tag="wa", name=f"wa_{tag}_{fg}")
                    nc.sync.dma_start(
                        out=w,
                        in_=waT[:, f0 : f0 + 512].rearrange("(a p) c -> p a c", p=128),
                    )
                    for tc2 in range(2):
                        for ftl in range(4):
                            ps = pool.tile(
                                [128, 512], f32, tag=ptag,
                                name=f"ps_{tag}_{fg}_{tc2}_{ftl}",
                            )
                            for ct in range(NCH):
                                nc.tensor.matmul(
                                    ps,
                                    w[:, ct, ftl * 128 : (ftl + 1) * 128],
                                    xt[ct][:, tc2 * 512 : (tc2 + 1) * 512],
                                    start=(ct == 0),
                                    stop=(ct == NCH - 1),
                                )
                            fti = fg * 4 + ftl
                            nc.vector.tensor_copy(
                                dst[fti][:, tc2 * 512 : (tc2 + 1) * 512], ps
                            )

            # fg0 halves of K and Q0 first, then V's weights, then the fg1
            # halves: attention(0) j=0 needs kt[0]/qt[0]/vt[*] (fg0+V) at
            # ~17us, while the fg1 halves are only needed from j==4 (~45us).
            project_T(kt, KOFF, "k", qp_pool, "psproj", (0,))
            # q-proj of set 0 on the op pool so it overlaps K/V-proj and lets
            # attention(0) start as early as possible
            project_T(qts[0], 0, "q0", op_pool, "psop", (0,))
            # mask constants aren't needed until the first diagonal block
            # (~40us in) — keep them off the head of the DMA queue
            nc.sync.dma_start(out=ident, in_=identD[:, :])
            nc.sync.dma_start(out=cmask, in_=cmaskD[:, :])

            # V in [token, feature] layout, features interleaved with a ones
            # column every 64 (each head's stationary V_aug slice is [128, 65]).
            wv = []
            for fg in range(2):
                f0 = VOFF + fg * 512
                w = wa_pool.tile([128, NCH, 512], bf16, tag="wa", name=f"wav_{fg}")
                nc.sync.dma_start(
                    out=w,
                    in_=waT[:, f0 : f0 + 512].rearrange("(a p) c -> p a c", p=128),
                )
                wv.append(w)
            # tt-major so each vt[tt] completes early: attention(0)'s first PV
            # matmuls need vt[0..3] long before the full V projection finishes
            for tt in range(NT):
                for fg in range(2):
                    w = wv[fg]
                    ps = op_pool.tile([128, 512], f32, tag="psop", name=f"psv_{fg}_{tt}")
                    for ct in range(NCH):
                        nc.tensor.matmul(
                            ps,
                            xt[ct][:, tt * 128 : (tt + 1) * 128],
                            w[:, ct, :],
                            start=(ct == 0),
                            stop=(ct == NCH - 1),
                        )
                    nc.vector.tensor_copy(
                        vtv[tt][:, fg * 8 : (fg + 1) * 8, 0:HD],
                        ps.rearrange("p (a b) -> p a b", b=HD),
                    )
            project_T(kt, KOFF, "kb", qp_pool, "psproj", (1,))
            project_T(qts[0], 0, "q0b", op_pool, "psop", (1,))

            wp_cache = {}

            def load_wp(pg, cc):
                """Dispatch the W_proj half-load one head-pair ahead of its
                chains, from the idle gpsimd queue so the 2.9us transfer never
                queues in front of the latency-critical normalize DMAs."""
                wp = wp_pool.tile([128, NCH, 512], bf16, tag="wp", name=f"wp{pg}_{cc}")
                nc.gpsimd.dma_start(
                    out=wp,
                    in_=wpT[
                        pg * C : (pg + 1) * C, cc * 512 : (cc + 1) * 512
                    ].rearrange("(a p) c -> p a c", p=128),
                )
                wp_cache[(pg, cc)] = wp

            def emit_outproj(pg, cc):
                """One cc half of out-projection for set pg, into osb.

                Emitted inside attention(pg+1)'s j loop so the scheduler
                keeps it as late PE filler there; set 3 is emitted after its
                own attention and alternates over all four idle PSUM pools.
                """
                pyt = yts[pg % 2]
                wp = wp3[cc] if pg == NQS - 1 else wp_cache.pop((pg, cc))
                for tt in range(NT):
                    if pg == NQS - 1:
                        pool2, tg = [
                            (op_pool, "psop"), (ypl, "ypl"),
                            (qp_pool, "psproj"), (ypl, "ypl"),
                        ][tt % 4]
                    else:
                        pool2, tg = op_pool, "psop"
                    ps = pool2.tile(
                        [128, 512], f32, tag=tg, name=f"psp{pg}_{cc}_{tt}"
                    )
                    for ftl in range(NCH):
                        nc.tensor.matmul(
                            ps,
                            pyt[ftl][:, tt * 128 : (tt + 1) * 128],
                            wp[:, ftl, :],
                            start=(ftl == 0),
                            stop=(ftl == NCH - 1),
                        )
                    dst = osb[tt][:, cc * 512 : (cc + 1) * 512]
                    if pg == 0:
                        nc.vector.tensor_copy(dst, ps)
                    else:
                        nc.vector.tensor_add(dst, dst, ps)
                    if pg == NQS - 1:
                        # stream each finished half out immediately so the
                        # store drain overlaps the remaining chains
                        nc.sync.dma_start(
                            out=out[tt * 128 : (tt + 1) * 128, cc * 512 : (cc + 1) * 512],
                            in_=dst,
                        )

            for g in range(NQS):
                qt = qts[g % 2]
                yt = yts[g % 2]
                if g > 0:
                    # for the last set, hold back the fg1 half: emitted inside
                    # the j loop below so the scheduler keeps it as PE filler
                    # for attention(3) (which has no q-proj of its own)
                    fgs = (0,) if g == NQS - 1 else (0, 1)
                    project_T(qt, g * C, f"q{g}", qp_pool, "psproj", fgs)
                if g == NQS - 1:
                    # both cc halves of W_proj for the last set, via the wp
                    # pool (free once set-2's chains consumed their weights),
                    # so the per-j out-projection of set 3 never waits on DMA
                    wp3 = []
                    for cc in range(2):
                        w3 = wa_pool.tile(
                            [128, NCH, 512], bf16, tag="wa", name=f"wp3_{cc}"
                        )
                        nc.gpsimd.dma_start(
                            out=w3,
                            in_=wpT[
                                3 * C : 4 * C, cc * 512 : (cc + 1) * 512
                            ].rearrange("(a p) c -> p a c", p=128),
                        )
                        wp3.append(w3)

                for j in range(NH // 2):  # head pairs (2j, 2j+1)
                    if g == NQS - 1 and j == 2:
                        project_T(qt, g * C, f"q{g}b", qp_pool, "psproj", (1,))
                    if g > 0 and j == 0:
                        load_wp(g - 1, 0)
                    if g > 0 and j == 1:
                        emit_outproj(g - 1, 0)
                    if g > 0 and j == 3:
                        load_wp(g - 1, 1)
                    # in the last window, hold the cc1 chains back to j==6 so
                    # they bridge the idle gap between the final normalize and
                    # the last projection (which used to trigger half-clock)
                    if g > 0 and j == (6 if g == NQS - 1 else 5):
                        emit_outproj(g - 1, 1)
                    for qc in range(2):  # 512-wide query chunks
                        nkt = 4 * qc + 4
                        yps = [
                            ypl.tile([128, 512], f32, tag="ypl", name=f"yp{g}_{j}_{qc}_{hh}")
                            for hh in range(2)
                        ]
                        # software-pipelined: QK(k2)+exp(k2) emitted one step
                        # ahead of PV(k2-1) so the PE never waits on ACT
                        pts = [None] * nkt
                        geom = []
                        for k2 in range(nkt):
                            qlo = max(qc * 512, k2 * 128)
                            wdt = qc * 512 + 512 - qlo
                            geom.append((qlo, wdt))
                        for k2 in range(nkt + 1):
                            if k2 < nkt:
                                qlo, wdt = geom[k2]
                                diag = k2 * 128 >= qc * 512
                                sp = ptmp.tile(
                                    [128, 1024], f32, tag="ptmp",
                                    name=f"sp{g}_{j}_{qc}_{k2}",
                                )
                                for hh in range(2):
                                    nc.tensor.matmul(
                                        sp[:, hh * 512 : hh * 512 + wdt],
                                        kt[j][hh * 64 : hh * 64 + 64, k2 * 128 : (k2 + 1) * 128],
                                        qt[j][hh * 64 : hh * 64 + 64, qlo : qlo + wdt],
                                        start=True,
                                        stop=not diag,
                                    )
                                if diag:
                                    # additive causal mask (0 / -1e30) on the
                                    # 128x128 diagonal blocks, applied on the
                                    # PE so the exp/PV deps stay single-engine
                                    for hh in range(2):
                                        nc.tensor.matmul(
                                            sp[:, hh * 512 : hh * 512 + 128],
                                            cmask,
                                            ident,
                                            start=False,
                                            stop=True,
                                            skip_group_check=True,
                                        )
                                pt = pt_pool.tile(
                                    [128, 2, 512], bf16, tag="pt",
                                    name=f"pt{g}_{j}_{qc}_{k2}",
                                )
                                nc.scalar.activation(
                                    pt[:, :, 0:wdt],
                                    sp.rearrange("p (a c) -> p a c", a=2)[:, :, 0:wdt],
                                    EXP,
                                    bias=0.0,
                                    scale=SCALE,
                                )
                                pts[k2] = pt
                            if k2 > 0:
                                qlo, wdt = geom[k2 - 1]
                                off = qlo - qc * 512
                                for hh in range(2):
                                    h = 2 * j + hh
                                    # 128-wide stationary slice (V_aug of head
                                    # h + spillover) -> FWL background load;
                                    # out rows 65-127 are unused garbage
                                    nc.tensor.matmul(
                                        yps[hh][:, off : off + wdt],
                                        vt[k2 - 1][:, h * (HD + 1) : h * (HD + 1) + 128],
                                        pts[k2 - 1][:, hh, 0:wdt],
                                        start=(k2 - 1 == 0),
                                        stop=(k2 - 1 == nkt - 1),
                                    )
                        # phase 1: BOTH casts first - each cast is the sole
                        # reader of its yps PSUM bank, and the bank release
                        # gates the next (j,qc) block's PV accumulation. The
                        # DMA-bounce-dependent ops go afterwards so a DRAM
                        # round trip never sits ahead of a cast in the queue.
                        yabs = []
                        for hh in range(2):
                            yab = yab_pool.tile(
                                [65, 512], bf16, tag="yab", name=f"yab{g}_{j}_{qc}_{hh}"
                            )
                            nc.vector.tensor_copy(yab, yps[hh][0:65, :])
                            yabs.append(yab)
                        # phase 2: spread the single-partition denominator row
                        # to [64, 8] lanes (SBUF->SBUF DMA), reciprocal, then
                        # DRAM hop for the partition-broadcast back
                        # DRAM bounce for the partition-broadcast: SBUF APs
                        # reject zero partition stride (verified), so the
                        # reciprocal rows must round-trip through DRAM
                        ci0 = ((g * 8 + j) * 2 + qc) * 2
                        # both den scatters dispatched before either recip so
                        # their DMA latencies overlap; the recips are the only
                        # DMA-waiting ops left ahead of the next block's casts
                        # in the vector queue
                        den2 = small_pool.tile(
                            [64, 16], bf16, tag="den64", name=f"den{g}_{j}_{qc}"
                        )
                        for hh in range(2):
                            nc.sync.dma_start(
                                out=den2[:, hh * 8 : (hh + 1) * 8],
                                in_=yabs[hh][64:65, :],
                            )
                        # ONE reciprocal for both heads (one fewer DMA-waiting
                        # op ahead of the next block's casts in the queue)
                        rec2 = small_pool.tile(
                            [64, 16], bf16, tag="rec64", name=f"rec{g}_{j}_{qc}"
                        )
                        with nc.allow_low_precision("bf16 softmax recip ok at 2e-2 tol"):
                            nc.vector.reciprocal(out=rec2, in_=den2)
                        for hh in range(2):
                            drow = rscr[ci0 + hh : ci0 + hh + 1, :]
                            nc.sync.dma_start(
                                out=drow.rearrange("a (b c) -> (a b) c", b=64),
                                in_=rec2[:, hh * 8 : (hh + 1) * 8],
                            )
                        # ONE partition-broadcast DMA for both heads' rows
                        # (halves the dispatch count on the sync queue)
                        drow2 = rscr[ci0 : ci0 + 2, :]
                        bc2 = small_pool.tile(
                            [64, 2, 512], bf16, tag="bcst", name=f"bcst{g}_{j}_{qc}"
                        )
                        nc.sync.dma_start(
                            out=bc2,
                            in_=bass.AP(
                                tensor=drow2.tensor,
                                offset=drow2.offset,
                                ap=[[0, 64]] + drow2.ap,
                            ),
                        )
                        # phase 3: normalize into yt (feeds the out-projection)
                        # on gpsimd: all-SBUF operands, ~10us of slack, and it
                        # keeps the bc2-waiting ops out of the vector queue
                        for hh in range(2):
                            nc.gpsimd.tensor_mul(
                                yt[j][hh * 64 : hh * 64 + 64, qc * 512 : qc * 512 + 512],
                                yabs[hh][0:64, :],
                                bc2[:, hh, :],
                            )

                if g == NQS - 1:
                    for cc in range(2):
                        emit_outproj(g, cc)



    nc.compile()
    return nc


def kernel(x, W_attn, W_proj, _trace=False):
    if "nc" not in _CACHE:
        _CACHE["nc"] = _build()
    nc = _CACHE["nc"]

    xT = np.ascontiguousarray(np.transpose(np.asarray(x, np.float32), (0, 2, 1))).astype(BF16)
    waT = np.ascontiguousarray(np.asarray(W_attn, np.float32).T).astype(BF16)
    wpT = np.ascontiguousarray(np.asarray(W_proj, np.float32).T).astype(BF16)
    ii = np.arange(128)
    ident = np.eye(128, dtype=np.float32).astype(BF16)
    # lhsT for the mask matmul: out[k,q] = cmaskT[q,k] = 0 if q>=k else -1e30
    cmaskT = (
        np.where(ii[:, None] >= ii[None, :], 0.0, -1e30)
        .astype(np.float32)
        .astype(BF16)
    )

    in_maps = [
        {"xT": xT[b], "waT": waT, "wpT": wpT, "ident": ident, "cmaskT": cmaskT}
        for b in range(B)
    ]
    res = run_bass_kernel_spmd(nc, in_maps, core_ids=list(range(B)), trace=_trace)
    LAST["exec_time_ns"] = res.exec_time_ns
    LAST["mean_exec_time_ns"] = res.mean_exec_time_ns
    LAST["results"] = res
    return np.stack([res.results[b]["out"] for b in range(B)]).astype(np.float32)



# revision 49
# speedup vs baseline: 1.0041x; 1.0041x over previous
"""Trainium2 Bass kernel for grouped-query causal self-attention.

Problem shapes (hardcoded): x [8,1024,1024] f32, W_attn [6144,1024] f32,
W_proj [1024,4096] f32. 16 heads, head_dim 64, 4 query sets sharing one K/V.

Sharding: data parallel over batch — one batch element per NeuronCore (8 cores).
No collectives needed.

Per-core algorithm (everything "transposed" = [feature, token] layout so no
on-device transposes are needed; x is pre-transposed on the host):
  1. qkvT tiles = W_attn @ x^T   (stationary = W_attn^T tile, moving = x^T)
     -> K^T [1024f, 1024t], Q_g^T per set, and V in normal [t, f] layout.
  2. Attention per (set g, head PAIR j=(2j,2j+1)), computed transposed:
     the two heads of a pair live in partitions 0-63 / 64-127 of the same
     kt/qt tile, so their 64-contraction QK^T matmuls run CONCURRENTLY on
     the PE via row tiling (tile_position (0,0) and (64,0)) — 2x QK rate.
        S^T[k, q] = K_tile^T-stationary @ Q^T-moving   (contraction = 64)
        causal: additive 0/-1e30 mask matmul (cmask@ident) accumulated onto
        the 128x128 diagonal blocks, on the PE so deps stay single-engine
        P^T = exp(S^T * scale)   one ACT op per k-tile covers both heads
        y_aug^T[d, q] = V_aug-stationary @ P^T-moving  (V augmented with a
        ones column -> row 64 of y_aug^T = softmax denominator for free;
        stationary slices padded to 128 cols for fast weight load)
        normalize: denominator row spread to [64,8] lanes via SBUF->SBUF DMA,
        bf16 reciprocal, DRAM hop for partition-broadcast, multiply (bf16).
  3. out = combined @ W_proj^T accumulated over sets in bf16 (osb), moving =
     W_proj^T streamed from DRAM in batched [128,8,512] loads.
dtypes: bf16 operands for matmuls (fp32 PSUM accumulate), fp32 softmax
denominator path, bf16 output accumulation (host converts to f32).

Scheduling notes (measured on HW, ~30us better than the naive ordering):
  - out-proj(g) is emitted inside attention(g+1)'s head-pair loop (cc=0 at
    j==1, cc=1 at j==5, or j==6 in the last window to bridge the gap into
    the final projection) and q-proj(3)'s second half at j==2 of
    attention(3), so the list scheduler holds PE filler back for the
    late-window stalls instead of draining it all at window start. The wp
    weight halves are dispatched one head-pair ahead of their chains, from
    the idle gpsimd queue so the 2.9us transfers never queue in front of
    the latency-critical normalize DMAs on sync.
  - set 3's W_proj tiles are prefetched through the wa pool from the idle
    gpsimd DMA queue (a pool-slot wait there blocks nothing), so the final
    projection never waits on DMA; it alternates over all four PSUM pools
    (op/ypl/qp/ypl, all idle by then) and streams each osb half to DRAM
    inline.
  - the softmax normalize is phased: both PSUM-releasing casts are emitted
    before any DMA-bounce-dependent op, so the ypl banks recycle without
    waiting on a DRAM round trip (the release gates the next block's PV).
  - x^T is loaded as ONE batched [128,8,1024] DMA (one dispatch, not 8);
    the startup weight loads go K-fg0, Q0-fg0, V, then K-fg1/Q0-fg1 (the
    fg1 halves aren't needed until j==4, while V gates the first PV), with
    wa bufs=4 so Q0-fg1's ring slot isn't held by a V load.
  - the two per-block reciprocal broadcasts are ONE [64,2,512] DMA; both
    den scatters land in one [64,16] tile feeding ONE reciprocal (their
    latencies overlap), and the normalize multiplies run on gpsimd
    (all-SBUF operands, ~10us slack) so no DMA-waiting op ever sits ahead
    of the next block's PSUM-releasing casts in the vector queue.
  - the wa/x weight DMAs stay on the sync queue: dispatching them from busy
    engine queues (scalar) head-of-line-blocks the exp ACTs behind them.
"""

import math

import ml_dtypes
import numpy as np

import concourse.bacc as bacc
import concourse.bass as bass
import concourse.mybir as mybir
import concourse.tile as tile
from concourse.bass_utils import run_bass_kernel_spmd

BF16 = ml_dtypes.bfloat16

B, T, C = 8, 1024, 1024
NH, HD, NQS = 16, 64, 4
SCALE = 1.0 / math.sqrt(HD)
NT = T // 128  # token tiles
NCH = C // 128  # channel tiles
KOFF = NQS * C  # 4096: K rows in W_attn
VOFF = (NQS + 1) * C  # 5120: V rows in W_attn

_CACHE = {}
LAST = {}  # exec_time_ns etc for test harness


def _build():
    f32 = mybir.dt.float32
    bf16 = mybir.dt.bfloat16
    EXP = mybir.ActivationFunctionType.Exp

    nc = bacc.Bacc()
    xT = nc.declare_dram_parameter("xT", [C, T], bf16, isOutput=False)
    waT = nc.declare_dram_parameter("waT", [C, 6 * C], bf16, isOutput=False)
    wpT = nc.declare_dram_parameter("wpT", [NQS * C, C], bf16, isOutput=False)
    identD = nc.declare_dram_parameter("ident", [128, 128], bf16, isOutput=False)
    cmaskD = nc.declare_dram_parameter("cmaskT", [128, 128], bf16, isOutput=False)
    out = nc.declare_dram_parameter("out", [T, C], bf16, isOutput=True)
    # DRAM bounce rows for the partition-broadcast of softmax reciprocals.
    rscr = nc.dram_tensor("rscr", [128, 512], bf16)

    with tile.TileContext(nc) as tc:
        with (
            tc.tile_pool(name="res", bufs=1) as res,
            tc.tile_pool(name="wa", bufs=4) as wa_pool,
            tc.tile_pool(name="wp", bufs=2) as wp_pool,
            tc.tile_pool(name="pt", bufs=6) as pt_pool,
            tc.tile_pool(name="yab", bufs=8) as yab_pool,
            tc.tile_pool(name="small", bufs=4) as small_pool,
            tc.tile_pool(name="qp", bufs=1, space="PSUM") as qp_pool,
            tc.tile_pool(name="op", bufs=1, space="PSUM") as op_pool,
            tc.tile_pool(name="ypl", bufs=2, space="PSUM") as ypl,
            tc.tile_pool(name="ptmp", bufs=2, space="PSUM") as ptmp,
        ):
            xtb = res.tile([128, NCH, T], bf16, tag="xtb", name="xtb")
            xt = [xtb[:, i, :] for i in range(NCH)]
            kt = [res.tile([128, T], bf16, tag=f"kt{i}", name=f"kt{i}") for i in range(NCH)]
            # [NH, 65] V_aug per head + 64 pad cols so every head h has a full
            # 128-wide stationary slice at offset h*65 (FWL-eligible LDW)
            vt = [res.tile([128, NH * (HD + 1) + 64], bf16, tag=f"vt{i}", name=f"vt{i}") for i in range(NT)]
            # double-buffered across query-set parity so q-proj(g+1) overlaps
            # attention(g), and out-proj(g) overlaps attention(g+1)
            qts = [
                [res.tile([128, T], bf16, tag=f"qt{p}_{i}", name=f"qt{p}_{i}") for i in range(NCH)]
                for p in range(2)
            ]
            yts = [
                [res.tile([128, T], bf16, tag=f"yt{p}_{i}", name=f"yt{p}_{i}") for i in range(NCH)]
                for p in range(2)
            ]
            osb = [res.tile([128, C], bf16, tag=f"osb{i}", name=f"osb{i}") for i in range(NT)]

            ident = res.tile([128, 128], bf16, tag="ident", name="ident")
            cmask = res.tile([128, 128], bf16, tag="cmask", name="cmask")
            vtv = [
                vt[tt][:, 0 : NH * (HD + 1)].rearrange("p (a b) -> p a b", b=HD + 1)
                for tt in range(NT)
            ]
            # x tiles first: they gate the first projection matmuls (the Sync
            # DMA queue is serial, so emission order = critical-path order)
            nc.sync.dma_start(out=xtb, in_=xT.rearrange("(a p) t -> p a t", p=128))
            # ones column + pad via on-chip memset instead of strided DMAs
            # (the DMA version cost ~1.4us apiece at the head of the queue)
            for tt in range(NT):
                nc.gpsimd.memset(vt[tt][:, NH * (HD + 1) :], 0.0)
                nc.gpsimd.memset(vtv[tt][:, :, HD : HD + 1], 1.0)

            def project_T(dst, fbase, tag, pool, ptag, fgs=(0, 1)):
                """dst[i][f_local, t] = (x @ W_attn.T).T rows fbase..fbase+1024."""
                for fg in fgs:  # 512-wide feature groups
                    f0 = fbase + fg * 512
                    w = wa_pool.tile([128, NCH, 512], bf16, tag="wa", name=f"wa_{tag}_{fg}")
                    nc.sync.dma_start(
                        out=w,
                        in_=waT[:, f0 : f0 + 512].rearrange("(a p) c -> p a c", p=128),
                    )
                    for tc2 in range(2):
                        for ftl in range(4):
                            ps = pool.tile(
                                [128, 512], f32, tag=ptag,
                                name=f"ps_{tag}_{fg}_{tc2}_{ftl}",
                            )
                            for ct in range(NCH):
                                nc.tensor.matmul(
                                    ps,
                                    w[:, ct, ftl * 128 : (ftl + 1) * 128],
                                    xt[ct][:, tc2 * 512 : (tc2 + 1) * 512],
                                    start=(ct == 0),
                                    stop=(ct == NCH - 1),
                                )
                            fti = fg * 4 + ftl
                            nc.vector.tensor_copy(
                                dst[fti][:, tc2 * 512 : (tc2 + 1) * 512], ps
                            )

            # fg0 halves of K and Q0 first, then V's weights, then the fg1
            # halves: attention(0) j=0 needs kt[0]/qt[0]/vt[*] (fg0+V) at
            # ~17us, while the fg1 halves are only needed from j==4 (~45us).
            project_T(kt, KOFF, "k", qp_pool, "psproj", (0,))
            # q-proj of set 0 on the op pool so it overlaps K/V-proj and lets
            # attention(0) start as early as possible
            project_T(qts[0], 0, "q0", op_pool, "psop", (0,))
            # mask constants aren't needed until the first diagonal block
            # (~40us in) — keep them off the head of the DMA queue
            nc.sync.dma_start(out=ident, in_=identD[:, :])
            nc.sync.dma_start(out=cmask, in_=cmaskD[:, :])

            # V in [token, feature] layout, features interleaved with a ones
            # column every 64 (each head's stationary V_aug slice is [128, 65]).
            wv = []
            for fg in range(2):
                f0 = VOFF + fg * 512
                w = wa_pool.tile([128, NCH, 512], bf16, tag="wa", name=f"wav_{fg}")
                nc.sync.dma_start(
                    out=w,
                    in_=waT[:, f0 : f0 + 512].rearrange("(a p) c -> p a c", p=128),
                )
                wv.append(w)
            # tt-major so each vt[tt] completes early: attention(0)'s first PV
            # matmuls need vt[0..3] long before the full V projection finishes
            for tt in range(NT):
                for fg in range(2):
                    w = wv[fg]
                    ps = op_pool.tile([128, 512], f32, tag="psop", name=f"psv_{fg}_{tt}")
                    for ct in range(NCH):
                        nc.tensor.matmul(
                            ps,
                            xt[ct][:, tt * 128 : (tt + 1) * 128],
                            w[:, ct, :],
                            start=(ct == 0),
                            stop=(ct == NCH - 1),
                        )
                    nc.vector.tensor_copy(
                        vtv[tt][:, fg * 8 : (fg + 1) * 8, 0:HD],
                        ps.rearrange("p (a b) -> p a b", b=HD),
                    )
            project_T(kt, KOFF, "kb", qp_pool, "psproj", (1,))
            project_T(qts[0], 0, "q0b", op_pool, "psop", (1,))

            wp_cache = {}

            def load_wp(pg, cc):
                """Dispatch the W_proj half-load one head-pair ahead of its
                chains, from the idle gpsimd queue so the 2.9us transfer never
                queues in front of the latency-critical normalize DMAs."""
                wp = wp_pool.tile([128, NCH, 512], bf16, tag="wp", name=f"wp{pg}_{cc}")
                nc.gpsimd.dma_start(
                    out=wp,
                    in_=wpT[
                        pg * C : (pg + 1) * C, cc * 512 : (cc + 1) * 512
                    ].rearrange("(a p) c -> p a c", p=128),
                )
                wp_cache[(pg, cc)] = wp

            def emit_outproj(pg, cc):
                """One cc half of out-projection for set pg, into osb.

                Emitted inside attention(pg+1)'s j loop so the scheduler
                keeps it as late PE filler there; set 3 is emitted after its
                own attention and alternates over all four idle PSUM pools.
                """
                pyt = yts[pg % 2]
                wp = wp3[cc] if pg == NQS - 1 else wp_cache.pop((pg, cc))
                for tt in range(NT):
                    if pg == NQS - 1:
                        pool2, tg = [
                            (op_pool, "psop"), (ypl, "ypl"),
                            (qp_pool, "psproj"), (ypl, "ypl"),
                        ][tt % 4]
                    else:
                        pool2, tg = op_pool, "psop"
                    ps = pool2.tile(
                        [128, 512], f32, tag=tg, name=f"psp{pg}_{cc}_{tt}"
                    )
                    for ftl in range(NCH):
                        nc.tensor.matmul(
                            ps,
                            pyt[ftl][:, tt * 128 : (tt + 1) * 128],
                            wp[:, ftl, :],
                            start=(ftl == 0),
                            stop=(ftl == NCH - 1),
                        )
                    dst = osb[tt][:, cc * 512 : (cc + 1) * 512]
                    if pg == 0:
                        nc.vector.tensor_copy(dst, ps)
                    else:
                        nc.vector.tensor_add(dst, dst, ps)
                    if pg == NQS - 1:
                        # stream each finished half out immediately so the
                        # store drain overlaps the remaining chains
                        nc.sync.dma_start(
                            out=out[tt * 128 : (tt + 1) * 128, cc * 512 : (cc + 1) * 512],
                            in_=dst,
                        )

            for g in range(NQS):
                qt = qts[g % 2]
                yt = yts[g % 2]
                if g > 0:
                    # for the last set, hold back the fg1 half: emitted inside
                    # the j loop below so the scheduler keeps it as PE filler
                    # for attention(3) (which has no q-proj of its own)
                    fgs = (0,) if g == NQS - 1 else (0, 1)
                    project_T(qt, g * C, f"q{g}", qp_pool, "psproj", fgs)
                if g == NQS - 1:
                    # both cc halves of W_proj for the last set, via the wp
                    # pool (free once set-2's chains consumed their weights),
                    # so the per-j out-projection of set 3 never waits on DMA
                    wp3 = []
                    for cc in range(2):
                        w3 = wa_pool.tile(
                            [128, NCH, 512], bf16, tag="wa", name=f"wp3_{cc}"
                        )
                        nc.gpsimd.dma_start(
                            out=w3,
                            in_=wpT[
                                3 * C : 4 * C, cc * 512 : (cc + 1) * 512
                            ].rearrange("(a p) c -> p a c", p=128),
                        )
                        wp3.append(w3)

                for j in range(NH // 2):  # head pairs (2j, 2j+1)
                    if g == NQS - 1 and j == 2:
                        project_T(qt, g * C, f"q{g}b", qp_pool, "psproj", (1,))
                    if g > 0 and j == 0:
                        load_wp(g - 1, 0)
                    if g > 0 and j == 1:
                        emit_outproj(g - 1, 0)
                    if g > 0 and j == 3:
                        load_wp(g - 1, 1)
                    # in the last window, hold the cc1 chains back to j==6 so
                    # they bridge the idle gap between the final normalize and
                    # the last projection (which used to trigger half-clock)
                    if g > 0 and j == (6 if g == NQS - 1 else 5):
                        emit_outproj(g - 1, 1)
                    for qc in range(2):  # 512-wide query chunks
                        nkt = 4 * qc + 4
                        yps = [
                            ypl.tile([128, 512], f32, tag="ypl", name=f"yp{g}_{j}_{qc}_{hh}")
                            for hh in range(2)
                        ]
                        # software-pipelined: QK(k2)+exp(k2) emitted one step
                        # ahead of PV(k2-1) so the PE never waits on ACT
                        pts = [None] * nkt
                        geom = []
                        for k2 in range(nkt):
                            qlo = max(qc * 512, k2 * 128)
                            wdt = qc * 512 + 512 - qlo
                            geom.append((qlo, wdt))
                        for k2 in range(nkt + 1):
                            if k2 < nkt:
                                qlo, wdt = geom[k2]
                                diag = k2 * 128 >= qc * 512
                                sp = ptmp.tile(
                                    [128, 1024], f32, tag="ptmp",
                                    name=f"sp{g}_{j}_{qc}_{k2}",
                                )
                                for hh in range(2):
                                    nc.tensor.matmul(
                                        sp[:, hh * 512 : hh * 512 + wdt],
                                        kt[j][hh * 64 : hh * 64 + 64, k2 * 128 : (k2 + 1) * 128],
                                        qt[j][hh * 64 : hh * 64 + 64, qlo : qlo + wdt],
                                        start=True,
                                        stop=not diag,
                                    )
                                if diag:
                                    # additive causal mask (0 / -1e30) on the
                                    # 128x128 diagonal blocks, applied on the
                                    # PE so the exp/PV deps stay single-engine
                                    for hh in range(2):
                                        nc.tensor.matmul(
                                            sp[:, hh * 512 : hh * 512 + 128],
                                            cmask,
                                            ident,
                                            start=False,
                                            stop=True,
                                            skip_group_check=True,
                                        )
                                pt = pt_pool.tile(
                                    [128, 2, 512], bf16, tag="pt",
                                    name=f"pt{g}_{j}_{qc}_{k2}",
                                )
                                nc.scalar.activation(
                                    pt[:, :, 0:wdt],
                                    sp.rearrange("p (a c) -> p a c", a=2)[:, :, 0:wdt],
                                    EXP,
                                    bias=0.0,
                                    scale=SCALE,
                                )
                                pts[k2] = pt
                            if k2 > 0:
                                qlo, wdt = geom[k2 - 1]
                                off = qlo - qc * 512
                                for hh in range(2):
                                    h = 2 * j + hh
                                    # 128-wide stationary slice (V_aug of head
                                    # h + spillover) -> FWL background load;
                                    # out rows 65-127 are unused garbage
                                    nc.tensor.matmul(
                                        yps[hh][:, off : off + wdt],
                                        vt[k2 - 1][:, h * (HD + 1) : h * (HD + 1) + 128],
                                        pts[k2 - 1][:, hh, 0:wdt],
                                        start=(k2 - 1 == 0),
                                        stop=(k2 - 1 == nkt - 1),
                                    )
                        # phase 1: BOTH casts first - each cast is the sole
                        # reader of its yps PSUM bank, and the bank release
                        # gates the next (j,qc) block's PV accumulation. The
                        # DMA-bounce-dependent ops go afterwards so a DRAM
                        # round trip never sits ahead of a cast in the queue.
                        yabs = []
                        for hh in range(2):
                            yab = yab_pool.tile(
                                [65, 512], bf16, tag="yab", name=f"yab{g}_{j}_{qc}_{hh}"
                            )
                            nc.vector.tensor_copy(yab, yps[hh][0:65, :])
                            yabs.append(yab)
                        # phase 2: spread the single-partition denominator row
                        # to [64, 8] lanes (SBUF->SBUF DMA), reciprocal, then
                        # DRAM hop for the partition-broadcast back
                        # DRAM bounce for the partition-broadcast: SBUF APs
                        # reject zero partition stride (verified), so the
                        # reciprocal rows must round-trip through DRAM
                        ci0 = ((g * 8 + j) * 2 + qc) * 2
                        # both den scatters dispatched before either recip so
                        # their DMA latencies overlap; the recips are the only
                        # DMA-waiting ops left ahead of the next block's casts
                        # in the vector queue
                        den2 = small_pool.tile(
                            [64, 16], bf16, tag="den64", name=f"den{g}_{j}_{qc}"
                        )
                        for hh in range(2):
                            nc.sync.dma_start(
                                out=den2[:, hh * 8 : (hh + 1) * 8],
                                in_=yabs[hh][64:65, :],
                            )
                        # ONE reciprocal for both heads (one fewer DMA-waiting
                        # op ahead of the next block's casts in the queue)
                        rec2 = small_pool.tile(
                            [64, 16], bf16, tag="rec64", name=f"rec{g}_{j}_{qc}"
                        )
                        with nc.allow_low_precision("bf16 softmax recip ok at 2e-2 tol"):
                            nc.vector.reciprocal(out=rec2, in_=den2)
                        for hh in range(2):
                            drow = rscr[ci0 + hh : ci0 + hh + 1, :]
                            nc.sync.dma_start(
                                out=drow.rearrange("a (b c) -> (a b) c", b=64),
                                in_=rec2[:, hh * 8 : (hh + 1) * 8],
                            )
                        # ONE partition-broadcast DMA for both heads' rows
                        # (halves the dispatch count on the sync queue)
                        drow2 = rscr[ci0 : ci0 + 2, :]
                        bc2 = small_pool.tile(
                            [64, 2, 512], bf16, tag="bcst", name=f"bcst{g}_{j}_{qc}"
                        )
                        nc.sync.dma_start(
                            out=bc2,
                            in_=bass.AP(
                                tensor=drow2.tensor,
                                offset=drow2.offset,
                                ap=[[0, 64]] + drow2.ap,
                            ),
                        )
                        # phase 3: normalize into yt (feeds the out-projection)
                        # on gpsimd: all-SBUF operands, ~10us of slack, and it
                        # keeps the bc2-waiting ops out of the vector queue.
                        # Exception: the LAST block sits on the tail critical
                        # path (final chains' ftl=7 wait on it) and vector is
                        # idle there - avoid gpsimd's ~1us Q7 launch latency.
                        meng = (
                            nc.vector if (g == NQS - 1 and j == NH // 2 - 1)
                            else nc.gpsimd
                        )
                        for hh in range(2):
                            meng.tensor_mul(
                                yt[j][hh * 64 : hh * 64 + 64, qc * 512 : qc * 512 + 512],
                                yabs[hh][0:64, :],
                                bc2[:, hh, :],
                            )

                if g == NQS - 1:
                    for cc in range(2):
                        emit_outproj(g, cc)



    nc.compile()
    return nc


def kernel(x, W_attn, W_proj, _trace=False):
    if "nc" not in _CACHE:
        _CACHE["nc"] = _build()
    nc = _CACHE["nc"]

    xT = np.ascontiguousarray(np.transpose(np.asarray(x, np.float32), (0, 2, 1))).astype(BF16)
    waT = np.ascontiguousarray(np.asarray(W_attn, np.float32).T).astype(BF16)
    wpT = np.ascontiguousarray(np.asarray(W_proj, np.float32).T).astype(BF16)
    ii = np.arange(128)
    ident = np.eye(128, dtype=np.float32).astype(BF16)
    # lhsT for the mask matmul: out[k,q] = cmaskT[q,k] = 0 if q>=k else -1e30
    cmaskT = (
        np.where(ii[:, None] >= ii[None, :], 0.0, -1e30)
        .astype(np.float32)
        .astype(BF16)
    )

    in_maps = [
        {"xT": xT[b], "waT": waT, "wpT": wpT, "ident": ident, "cmaskT": cmaskT}
        for b in range(B)
    ]
    res = run_bass_kernel_spmd(nc, in_maps, core_ids=list(range(B)), trace=_trace)
    LAST["exec_time_ns"] = res.exec_time_ns
    LAST["mean_exec_time_ns"] = res.mean_exec_time_ns
    LAST["results"] = res
    return np.stack([res.results[b]["out"] for b in range(B)]).astype(np.float32)



# revision 50
# speedup vs baseline: 1.0098x; 1.0056x over previous
"""Trainium2 Bass kernel for grouped-query causal self-attention.

Problem shapes (hardcoded): x [8,1024,1024] f32, W_attn [6144,1024] f32,
W_proj [1024,4096] f32. 16 heads, head_dim 64, 4 query sets sharing one K/V.

Sharding: data parallel over batch — one batch element per NeuronCore (8 cores).
No collectives needed.

Per-core algorithm (everything "transposed" = [feature, token] layout so no
on-device transposes are needed; x is pre-transposed on the host):
  1. qkvT tiles = W_attn @ x^T   (stationary = W_attn^T tile, moving = x^T)
     -> K^T [1024f, 1024t], Q_g^T per set, and V in normal [t, f] layout.
  2. Attention per (set g, head PAIR j=(2j,2j+1)), computed transposed:
     the two heads of a pair live in partitions 0-63 / 64-127 of the same
     kt/qt tile, so their 64-contraction QK^T matmuls run CONCURRENTLY on
     the PE via row tiling (tile_position (0,0) and (64,0)) — 2x QK rate.
        S^T[k, q] = K_tile^T-stationary @ Q^T-moving   (contraction = 64)
        causal: additive 0/-1e30 mask matmul (cmask@ident) accumulated onto
        the 128x128 diagonal blocks, on the PE so deps stay single-engine
        P^T = exp(S^T * scale)   one ACT op per k-tile covers both heads
        y_aug^T[d, q] = V_aug-stationary @ P^T-moving  (V augmented with a
        ones column -> row 64 of y_aug^T = softmax denominator for free;
        stationary slices padded to 128 cols for fast weight load)
        normalize: denominator row spread to [64,8] lanes via SBUF->SBUF DMA,
        bf16 reciprocal, DRAM hop for partition-broadcast, multiply (bf16).
  3. out = combined @ W_proj^T accumulated over sets in bf16 (osb), moving =
     W_proj^T streamed from DRAM in batched [128,8,512] loads.
dtypes: bf16 operands for matmuls (fp32 PSUM accumulate), fp32 softmax
denominator path, bf16 output accumulation (host converts to f32).

Scheduling notes (measured on HW, ~30us better than the naive ordering):
  - out-proj(g) is emitted inside attention(g+1)'s head-pair loop (cc=0 at
    j==1, cc=1 at j==5, or j==6 in the last window to bridge the gap into
    the final projection) and q-proj(3)'s second half at j==2 of
    attention(3), so the list scheduler holds PE filler back for the
    late-window stalls instead of draining it all at window start. The wp
    weight halves are dispatched one head-pair ahead of their chains, from
    the idle gpsimd queue so the 2.9us transfers never queue in front of
    the latency-critical normalize DMAs on sync.
  - set 3's W_proj tiles are prefetched through the wa pool from the idle
    gpsimd DMA queue (a pool-slot wait there blocks nothing), so the final
    projection never waits on DMA; it alternates over all four PSUM pools
    (op/ypl/qp/ypl, all idle by then) and streams each osb half to DRAM
    inline.
  - the softmax normalize is phased: both PSUM-releasing casts are emitted
    before any DMA-bounce-dependent op, so the ypl banks recycle without
    waiting on a DRAM round trip (the release gates the next block's PV).
  - x^T is loaded as ONE batched [128,8,1024] DMA (one dispatch, not 8);
    the startup weight loads go K-fg0, Q0-fg0, V, then K-fg1/Q0-fg1 (the
    fg1 halves aren't needed until j==4, while V gates the first PV), with
    wa bufs=4 so Q0-fg1's ring slot isn't held by a V load.
  - the two per-block reciprocal broadcasts are ONE [64,2,512] DMA; both
    den scatters land in one [64,16] tile feeding ONE reciprocal (their
    latencies overlap), and the normalize multiplies run on gpsimd
    (all-SBUF operands, ~10us slack) so no DMA-waiting op ever sits ahead
    of the next block's PSUM-releasing casts in the vector queue.
  - the wa/x weight DMAs stay on the sync queue: dispatching them from busy
    engine queues (scalar) head-of-line-blocks the exp ACTs behind them.
"""

import math

import ml_dtypes
import numpy as np

import concourse.bacc as bacc
import concourse.bass as bass
import concourse.mybir as mybir
import concourse.tile as tile
from concourse.bass_utils import run_bass_kernel_spmd

BF16 = ml_dtypes.bfloat16

B, T, C = 8, 1024, 1024
NH, HD, NQS = 16, 64, 4
SCALE = 1.0 / math.sqrt(HD)
NT = T // 128  # token tiles
NCH = C // 128  # channel tiles
KOFF = NQS * C  # 4096: K rows in W_attn
VOFF = (NQS + 1) * C  # 5120: V rows in W_attn

_CACHE = {}
LAST = {}  # exec_time_ns etc for test harness


def _build():
    f32 = mybir.dt.float32
    bf16 = mybir.dt.bfloat16
    EXP = mybir.ActivationFunctionType.Exp

    nc = bacc.Bacc()
    xT = nc.declare_dram_parameter("xT", [C, T], bf16, isOutput=False)
    waT = nc.declare_dram_parameter("waT", [C, 6 * C], bf16, isOutput=False)
    wpT = nc.declare_dram_parameter("wpT", [NQS * C, C], bf16, isOutput=False)
    identD = nc.declare_dram_parameter("ident", [128, 128], bf16, isOutput=False)
    cmaskD = nc.declare_dram_parameter("cmaskT", [128, 128], bf16, isOutput=False)
    out = nc.declare_dram_parameter("out", [T, C], bf16, isOutput=True)
    # DRAM bounce rows for the partition-broadcast of softmax reciprocals.
    rscr = nc.dram_tensor("rscr", [128, 512], bf16)

    with tile.TileContext(nc) as tc:
        with (
            tc.tile_pool(name="res", bufs=1) as res,
            tc.tile_pool(name="wa", bufs=4) as wa_pool,
            tc.tile_pool(name="wp", bufs=2) as wp_pool,
            tc.tile_pool(name="pt", bufs=6) as pt_pool,
            tc.tile_pool(name="yab", bufs=8) as yab_pool,
            tc.tile_pool(name="small", bufs=4) as small_pool,
            tc.tile_pool(name="qp", bufs=1, space="PSUM") as qp_pool,
            tc.tile_pool(name="op", bufs=1, space="PSUM") as op_pool,
            tc.tile_pool(name="ypl", bufs=2, space="PSUM") as ypl,
            tc.tile_pool(name="ptmp", bufs=2, space="PSUM") as ptmp,
        ):
            xtb = res.tile([128, NCH, T], bf16, tag="xtb", name="xtb")
            xt = [xtb[:, i, :] for i in range(NCH)]
            kt = [res.tile([128, T], bf16, tag=f"kt{i}", name=f"kt{i}") for i in range(NCH)]
            # [NH, 65] V_aug per head + 64 pad cols so every head h has a full
            # 128-wide stationary slice at offset h*65 (FWL-eligible LDW)
            vt = [res.tile([128, NH * (HD + 1) + 64], bf16, tag=f"vt{i}", name=f"vt{i}") for i in range(NT)]
            # double-buffered across query-set parity so q-proj(g+1) overlaps
            # attention(g), and out-proj(g) overlaps attention(g+1)
            qts = [
                [res.tile([128, T], bf16, tag=f"qt{p}_{i}", name=f"qt{p}_{i}") for i in range(NCH)]
                for p in range(2)
            ]
            yts = [
                [res.tile([128, T], bf16, tag=f"yt{p}_{i}", name=f"yt{p}_{i}") for i in range(NCH)]
                for p in range(2)
            ]
            osb = [res.tile([128, C], bf16, tag=f"osb{i}", name=f"osb{i}") for i in range(NT)]

            ident = res.tile([128, 128], bf16, tag="ident", name="ident")
            cmask = res.tile([128, 128], bf16, tag="cmask", name="cmask")
            vtv = [
                vt[tt][:, 0 : NH * (HD + 1)].rearrange("p (a b) -> p a b", b=HD + 1)
                for tt in range(NT)
            ]
            # x tiles first: they gate the first projection matmuls (the Sync
            # DMA queue is serial, so emission order = critical-path order)
            nc.sync.dma_start(out=xtb, in_=xT.rearrange("(a p) t -> p a t", p=128))
            # ones column + pad via on-chip memset instead of strided DMAs
            # (the DMA version cost ~1.4us apiece at the head of the queue)
            for tt in range(NT):
                nc.gpsimd.memset(vt[tt][:, NH * (HD + 1) :], 0.0)
                nc.gpsimd.memset(vtv[tt][:, :, HD : HD + 1], 1.0)

            def project_T(dst, fbase, tag, pool, ptag, fgs=(0, 1)):
                """dst[i][f_local, t] = (x @ W_attn.T).T rows fbase..fbase+1024."""
                for fg in fgs:  # 512-wide feature groups
                    f0 = fbase + fg * 512
                    w = wa_pool.tile([128, NCH, 512], bf16, tag="wa", name=f"wa_{tag}_{fg}")
                    nc.sync.dma_start(
                        out=w,
                        in_=waT[:, f0 : f0 + 512].rearrange("(a p) c -> p a c", p=128),
                    )
                    for tc2 in range(2):
                        for ftl in range(4):
                            ps = pool.tile(
                                [128, 512], f32, tag=ptag,
                                name=f"ps_{tag}_{fg}_{tc2}_{ftl}",
                            )
                            for ct in range(NCH):
                                nc.tensor.matmul(
                                    ps,
                                    w[:, ct, ftl * 128 : (ftl + 1) * 128],
                                    xt[ct][:, tc2 * 512 : (tc2 + 1) * 512],
                                    start=(ct == 0),
                                    stop=(ct == NCH - 1),
                                )
                            fti = fg * 4 + ftl
                            nc.vector.tensor_copy(
                                dst[fti][:, tc2 * 512 : (tc2 + 1) * 512], ps
                            )

            # fg0 halves of K and Q0 first, then V's weights, then the fg1
            # halves: attention(0) j=0 needs kt[0]/qt[0]/vt[*] (fg0+V) at
            # ~17us, while the fg1 halves are only needed from j==4 (~45us).
            project_T(kt, KOFF, "k", qp_pool, "psproj", (0,))
            # q-proj of set 0 on the op pool so it overlaps K/V-proj and lets
            # attention(0) start as early as possible
            project_T(qts[0], 0, "q0", op_pool, "psop", (0,))
            # mask constants aren't needed until the first diagonal block
            # (~40us in) — keep them off the head of the DMA queue
            nc.sync.dma_start(out=ident, in_=identD[:, :])
            nc.sync.dma_start(out=cmask, in_=cmaskD[:, :])

            # V in [token, feature] layout, features interleaved with a ones
            # column every 64 (each head's stationary V_aug slice is [128, 65]).
            wv = []
            for fg in range(2):
                f0 = VOFF + fg * 512
                w = wa_pool.tile([128, NCH, 512], bf16, tag="wa", name=f"wav_{fg}")
                nc.sync.dma_start(
                    out=w,
                    in_=waT[:, f0 : f0 + 512].rearrange("(a p) c -> p a c", p=128),
                )
                wv.append(w)
            # tt-major so each vt[tt] completes early: attention(0)'s first PV
            # matmuls need vt[0..3] long before the full V projection finishes
            for tt in range(NT):
                for fg in range(2):
                    w = wv[fg]
                    ps = op_pool.tile([128, 512], f32, tag="psop", name=f"psv_{fg}_{tt}")
                    for ct in range(NCH):
                        nc.tensor.matmul(
                            ps,
                            xt[ct][:, tt * 128 : (tt + 1) * 128],
                            w[:, ct, :],
                            start=(ct == 0),
                            stop=(ct == NCH - 1),
                        )
                    nc.vector.tensor_copy(
                        vtv[tt][:, fg * 8 : (fg + 1) * 8, 0:HD],
                        ps.rearrange("p (a b) -> p a b", b=HD),
                    )
            project_T(kt, KOFF, "kb", qp_pool, "psproj", (1,))
            project_T(qts[0], 0, "q0b", op_pool, "psop", (1,))

            wp_cache = {}

            def load_wp(pg, cc):
                """Dispatch the W_proj half-load one head-pair ahead of its
                chains, from the idle gpsimd queue so the 2.9us transfer never
                queues in front of the latency-critical normalize DMAs."""
                wp = wp_pool.tile([128, NCH, 512], bf16, tag="wp", name=f"wp{pg}_{cc}")
                nc.gpsimd.dma_start(
                    out=wp,
                    in_=wpT[
                        pg * C : (pg + 1) * C, cc * 512 : (cc + 1) * 512
                    ].rearrange("(a p) c -> p a c", p=128),
                )
                wp_cache[(pg, cc)] = wp

            def emit_outproj(pg, cc, tts=None):
                """One cc half of out-projection for set pg, into osb.

                Emitted inside attention(pg+1)'s j loop so the scheduler
                keeps it as late PE filler there; set 3 is emitted after its
                own attention and alternates over all four idle PSUM pools.
                """
                pyt = yts[pg % 2]
                wp = wp3[cc] if pg == NQS - 1 else wp_cache.get((pg, cc))
                for tt in (range(NT) if tts is None else tts):
                    if pg == NQS - 1:
                        pool2, tg = [
                            (op_pool, "psop"), (ypl, "ypl"),
                            (qp_pool, "psproj"), (ypl, "ypl"),
                        ][tt % 4]
                    else:
                        pool2, tg = op_pool, "psop"
                    ps = pool2.tile(
                        [128, 512], f32, tag=tg, name=f"psp{pg}_{cc}_{tt}"
                    )
                    for ftl in range(NCH):
                        nc.tensor.matmul(
                            ps,
                            pyt[ftl][:, tt * 128 : (tt + 1) * 128],
                            wp[:, ftl, :],
                            start=(ftl == 0),
                            stop=(ftl == NCH - 1),
                        )
                    dst = osb[tt][:, cc * 512 : (cc + 1) * 512]
                    if pg == 0:
                        nc.vector.tensor_copy(dst, ps)
                    else:
                        nc.vector.tensor_add(dst, dst, ps)
                    if pg == NQS - 1:
                        # stream each finished half out immediately so the
                        # store drain overlaps the remaining chains
                        nc.sync.dma_start(
                            out=out[tt * 128 : (tt + 1) * 128, cc * 512 : (cc + 1) * 512],
                            in_=dst,
                        )

            for g in range(NQS):
                qt = qts[g % 2]
                yt = yts[g % 2]
                if g > 0:
                    # for the last set, hold back the fg1 half: emitted inside
                    # the j loop below so the scheduler keeps it as PE filler
                    # for attention(3) (which has no q-proj of its own)
                    fgs = (0,) if g == NQS - 1 else (0, 1)
                    project_T(qt, g * C, f"q{g}", qp_pool, "psproj", fgs)
                if g == NQS - 1:
                    # both cc halves of W_proj for the last set, via the wp
                    # pool (free once set-2's chains consumed their weights),
                    # so the per-j out-projection of set 3 never waits on DMA
                    wp3 = []
                    for cc in range(2):
                        w3 = wa_pool.tile(
                            [128, NCH, 512], bf16, tag="wa", name=f"wp3_{cc}"
                        )
                        nc.gpsimd.dma_start(
                            out=w3,
                            in_=wpT[
                                3 * C : 4 * C, cc * 512 : (cc + 1) * 512
                            ].rearrange("(a p) c -> p a c", p=128),
                        )
                        wp3.append(w3)

                for j in range(NH // 2):  # head pairs (2j, 2j+1)
                    if g == NQS - 1 and j == 2:
                        project_T(qt, g * C, f"q{g}b", qp_pool, "psproj", (1,))
                    if g > 0 and j == 0:
                        load_wp(g - 1, 0)
                    if g > 0 and j == 1:
                        emit_outproj(g - 1, 0)
                    if g > 0 and j == 3:
                        load_wp(g - 1, 1)
                    # in the last window, split the cc1 chains across j==6
                    # and j==7 so they bridge BOTH observed idle regions (mid
                    # j6-7 and the final-normalize-to-last-projection gap);
                    # they read yt2, cold for a full window, so the late
                    # placement cannot block the in-order PE queue
                    if g > 0:
                        if g == NQS - 1:
                            if j == 6:
                                emit_outproj(g - 1, 1, range(0, NT // 2))
                            if j == 7:
                                emit_outproj(g - 1, 1, range(NT // 2, NT))
                        elif j == 5:
                            emit_outproj(g - 1, 1)
                    for qc in range(2):  # 512-wide query chunks
                        nkt = 4 * qc + 4
                        yps = [
                            ypl.tile([128, 512], f32, tag="ypl", name=f"yp{g}_{j}_{qc}_{hh}")
                            for hh in range(2)
                        ]
                        # software-pipelined: QK(k2)+exp(k2) emitted one step
                        # ahead of PV(k2-1) so the PE never waits on ACT
                        pts = [None] * nkt
                        geom = []
                        for k2 in range(nkt):
                            qlo = max(qc * 512, k2 * 128)
                            wdt = qc * 512 + 512 - qlo
                            geom.append((qlo, wdt))
                        for k2 in range(nkt + 1):
                            if k2 < nkt:
                                qlo, wdt = geom[k2]
                                diag = k2 * 128 >= qc * 512
                                sp = ptmp.tile(
                                    [128, 1024], f32, tag="ptmp",
                                    name=f"sp{g}_{j}_{qc}_{k2}",
                                )
                                for hh in range(2):
                                    nc.tensor.matmul(
                                        sp[:, hh * 512 : hh * 512 + wdt],
                                        kt[j][hh * 64 : hh * 64 + 64, k2 * 128 : (k2 + 1) * 128],
                                        qt[j][hh * 64 : hh * 64 + 64, qlo : qlo + wdt],
                                        start=True,
                                        stop=not diag,
                                    )
                                if diag:
                                    # additive causal mask (0 / -1e30) on the
                                    # 128x128 diagonal blocks, applied on the
                                    # PE so the exp/PV deps stay single-engine
                                    for hh in range(2):
                                        nc.tensor.matmul(
                                            sp[:, hh * 512 : hh * 512 + 128],
                                            cmask,
                                            ident,
                                            start=False,
                                            stop=True,
                                            skip_group_check=True,
                                        )
                                pt = pt_pool.tile(
                                    [128, 2, 512], bf16, tag="pt",
                                    name=f"pt{g}_{j}_{qc}_{k2}",
                                )
                                nc.scalar.activation(
                                    pt[:, :, 0:wdt],
                                    sp.rearrange("p (a c) -> p a c", a=2)[:, :, 0:wdt],
                                    EXP,
                                    bias=0.0,
                                    scale=SCALE,
                                )
                                pts[k2] = pt
                            if k2 > 0:
                                qlo, wdt = geom[k2 - 1]
                                off = qlo - qc * 512
                                for hh in range(2):
                                    h = 2 * j + hh
                                    # 128-wide stationary slice (V_aug of head
                                    # h + spillover) -> FWL background load;
                                    # out rows 65-127 are unused garbage
                                    nc.tensor.matmul(
                                        yps[hh][:, off : off + wdt],
                                        vt[k2 - 1][:, h * (HD + 1) : h * (HD + 1) + 128],
                                        pts[k2 - 1][:, hh, 0:wdt],
                                        start=(k2 - 1 == 0),
                                        stop=(k2 - 1 == nkt - 1),
                                    )
                        # phase 1: BOTH casts first - each cast is the sole
                        # reader of its yps PSUM bank, and the bank release
                        # gates the next (j,qc) block's PV accumulation. The
                        # DMA-bounce-dependent ops go afterwards so a DRAM
                        # round trip never sits ahead of a cast in the queue.
                        yabs = []
                        for hh in range(2):
                            yab = yab_pool.tile(
                                [65, 512], bf16, tag="yab", name=f"yab{g}_{j}_{qc}_{hh}"
                            )
                            nc.vector.tensor_copy(yab, yps[hh][0:65, :])
                            yabs.append(yab)
                        # phase 2: spread the single-partition denominator row
                        # to [64, 8] lanes (SBUF->SBUF DMA), reciprocal, then
                        # DRAM hop for the partition-broadcast back
                        # DRAM bounce for the partition-broadcast: SBUF APs
                        # reject zero partition stride (verified), so the
                        # reciprocal rows must round-trip through DRAM
                        ci0 = ((g * 8 + j) * 2 + qc) * 2
                        # both den scatters dispatched before either recip so
                        # their DMA latencies overlap; the recips are the only
                        # DMA-waiting ops left ahead of the next block's casts
                        # in the vector queue
                        den2 = small_pool.tile(
                            [64, 16], bf16, tag="den64", name=f"den{g}_{j}_{qc}"
                        )
                        for hh in range(2):
                            nc.sync.dma_start(
                                out=den2[:, hh * 8 : (hh + 1) * 8],
                                in_=yabs[hh][64:65, :],
                            )
                        # ONE reciprocal for both heads (one fewer DMA-waiting
                        # op ahead of the next block's casts in the queue)
                        rec2 = small_pool.tile(
                            [64, 16], bf16, tag="rec64", name=f"rec{g}_{j}_{qc}"
                        )
                        with nc.allow_low_precision("bf16 softmax recip ok at 2e-2 tol"):
                            nc.vector.reciprocal(out=rec2, in_=den2)
                        for hh in range(2):
                            drow = rscr[ci0 + hh : ci0 + hh + 1, :]
                            nc.sync.dma_start(
                                out=drow.rearrange("a (b c) -> (a b) c", b=64),
                                in_=rec2[:, hh * 8 : (hh + 1) * 8],
                            )
                        # ONE partition-broadcast DMA for both heads' rows
                        # (halves the dispatch count on the sync queue)
                        drow2 = rscr[ci0 : ci0 + 2, :]
                        bc2 = small_pool.tile(
                            [64, 2, 512], bf16, tag="bcst", name=f"bcst{g}_{j}_{qc}"
                        )
                        nc.sync.dma_start(
                            out=bc2,
                            in_=bass.AP(
                                tensor=drow2.tensor,
                                offset=drow2.offset,
                                ap=[[0, 64]] + drow2.ap,
                            ),
                        )
                        # phase 3: normalize into yt (feeds the out-projection)
                        # on gpsimd: all-SBUF operands, ~10us of slack, and it
                        # keeps the bc2-waiting ops out of the vector queue.
                        # Exception: the LAST block sits on the tail critical
                        # path (final chains' ftl=7 wait on it) and vector is
                        # idle there - avoid gpsimd's ~1us Q7 launch latency.
                        meng = (
                            nc.vector if (g == NQS - 1 and j == NH // 2 - 1)
                            else nc.gpsimd
                        )
                        for hh in range(2):
                            meng.tensor_mul(
                                yt[j][hh * 64 : hh * 64 + 64, qc * 512 : qc * 512 + 512],
                                yabs[hh][0:64, :],
                                bc2[:, hh, :],
                            )

                if g == NQS - 1:
                    for cc in range(2):
                        emit_outproj(g, cc)



    nc.compile()
    return nc


def kernel(x, W_attn, W_proj, _trace=False):
    if "nc" not in _CACHE:
        _CACHE["nc"] = _build()
    nc = _CACHE["nc"]

    xT = np.ascontiguousarray(np.transpose(np.asarray(x, np.float32), (0, 2, 1))).astype(BF16)
    waT = np.ascontiguousarray(np.asarray(W_attn, np.float32).T).astype(BF16)
    wpT = np.ascontiguousarray(np.asarray(W_proj, np.float32).T).astype(BF16)
    ii = np.arange(128)
    ident = np.eye(128, dtype=np.float32).astype(BF16)
    # lhsT for the mask matmul: out[k,q] = cmaskT[q,k] = 0 if q>=k else -1e30
    cmaskT = (
        np.where(ii[:, None] >= ii[None, :], 0.0, -1e30)
        .astype(np.float32)
        .astype(BF16)
    )

    in_maps = [
        {"xT": xT[b], "waT": waT, "wpT": wpT, "ident": ident, "cmaskT": cmaskT}
        for b in range(B)
    ]
    res = run_bass_kernel_spmd(nc, in_maps, core_ids=list(range(B)), trace=_trace)
    LAST["exec_time_ns"] = res.exec_time_ns
    LAST["mean_exec_time_ns"] = res.mean_exec_time_ns
    LAST["results"] = res
    return np.stack([res.results[b]["out"] for b in range(B)]).astype(np.float32)



# revision 51
# speedup vs baseline: 1.0309x; 1.0210x over previous
"""Trainium2 Bass kernel for grouped-query causal self-attention.

Problem shapes (hardcoded): x [8,1024,1024] f32, W_attn [6144,1024] f32,
W_proj [1024,4096] f32. 16 heads, head_dim 64, 4 query sets sharing one K/V.

Sharding: data parallel over batch — one batch element per NeuronCore (8 cores).
No collectives needed.

Per-core algorithm (everything "transposed" = [feature, token] layout so no
on-device transposes are needed; x is pre-transposed on the host):
  1. qkvT tiles = W_attn @ x^T   (stationary = W_attn^T tile, moving = x^T)
     -> K^T [1024f, 1024t], Q_g^T per set, and V in normal [t, f] layout.
  2. Attention per (set g, head PAIR j=(2j,2j+1)), computed transposed:
     the two heads of a pair live in partitions 0-63 / 64-127 of the same
     kt/qt tile, so their 64-contraction QK^T matmuls run CONCURRENTLY on
     the PE via row tiling (tile_position (0,0) and (64,0)) — 2x QK rate.
        S^T[k, q] = K_tile^T-stationary @ Q^T-moving   (contraction = 64)
        causal: additive 0/-1e30 mask matmul (cmask@ident) accumulated onto
        the 128x128 diagonal blocks, on the PE so deps stay single-engine
        P^T = exp(S^T * scale)   one ACT op per k-tile covers both heads
        y_aug^T[d, q] = V_aug-stationary @ P^T-moving  (V augmented with a
        ones column -> row 64 of y_aug^T = softmax denominator for free;
        stationary slices padded to 128 cols for fast weight load)
        normalize: denominator row spread to [64,8] lanes via SBUF->SBUF DMA,
        bf16 reciprocal, DRAM hop for partition-broadcast, multiply (bf16).
  3. out = combined @ W_proj^T accumulated over sets in bf16 (osb), moving =
     W_proj^T streamed from DRAM in batched [128,8,512] loads.
dtypes: bf16 operands for matmuls (fp32 PSUM accumulate), fp32 softmax
denominator path, bf16 output accumulation (host converts to f32).

Scheduling notes (measured on HW, ~30us better than the naive ordering):
  - out-proj(g) is emitted inside attention(g+1)'s head-pair loop (cc=0 at
    j==1, cc=1 at j==5; in the last window cc=1 is split 4+4 across j==6
    and j==7 to bridge both late idle regions) and q-proj(3)'s second half at j==2 of
    attention(3), so the list scheduler holds PE filler back for the
    late-window stalls instead of draining it all at window start. The wp
    weight halves are dispatched one head-pair ahead of their chains, from
    the idle gpsimd queue so the 2.9us transfers never queue in front of
    the latency-critical normalize DMAs on sync.
  - set 3's W_proj tiles are prefetched through the wa pool from the idle
    gpsimd DMA queue (a pool-slot wait there blocks nothing), so the final
    projection never waits on DMA; it alternates over all four PSUM pools
    (op/ypl/qp/ypl, all idle by then) and streams each osb half to DRAM
    inline.
  - the softmax normalize is phased: both PSUM-releasing casts are emitted
    before any DMA-bounce-dependent op, so the ypl banks recycle without
    waiting on a DRAM round trip (the release gates the next block's PV).
  - x^T is loaded as ONE batched [128,8,1024] DMA (one dispatch, not 8);
    the startup weight loads go K-fg0, Q0-fg0, V, then K-fg1/Q0-fg1 (the
    fg1 halves aren't needed until j==4, while V gates the first PV), with
    wa bufs=4 so Q0-fg1's ring slot isn't held by a V load.
  - the two per-block reciprocal broadcasts are ONE [64,2,512] DMA; both
    den scatters land in one [64,16] tile feeding ONE reciprocal (their
    latencies overlap), and the normalize multiplies run on gpsimd
    (all-SBUF operands, ~10us slack) so no DMA-waiting op ever sits ahead
    of the next block's PSUM-releasing casts in the vector queue.
  - the wa/x weight DMAs stay on the sync queue: dispatching them from busy
    engine queues (scalar) head-of-line-blocks the exp ACTs behind them.
"""

import math

import ml_dtypes
import numpy as np

import concourse.bacc as bacc
import concourse.bass as bass
import concourse.mybir as mybir
import concourse.tile as tile
from concourse.bass_utils import run_bass_kernel_spmd

BF16 = ml_dtypes.bfloat16

B, T, C = 8, 1024, 1024
NH, HD, NQS = 16, 64, 4
SCALE = 1.0 / math.sqrt(HD)
NT = T // 128  # token tiles
NCH = C // 128  # channel tiles
KOFF = NQS * C  # 4096: K rows in W_attn
VOFF = (NQS + 1) * C  # 5120: V rows in W_attn

_CACHE = {}
LAST = {}  # exec_time_ns etc for test harness


def _build():
    f32 = mybir.dt.float32
    bf16 = mybir.dt.bfloat16
    EXP = mybir.ActivationFunctionType.Exp

    nc = bacc.Bacc()
    xT = nc.declare_dram_parameter("xT", [C, T], bf16, isOutput=False)
    waT = nc.declare_dram_parameter("waT", [C, 6 * C], bf16, isOutput=False)
    wpT = nc.declare_dram_parameter("wpT", [NQS * C, C], bf16, isOutput=False)
    identD = nc.declare_dram_parameter("ident", [128, 128], bf16, isOutput=False)
    cmaskD = nc.declare_dram_parameter("cmaskT", [128, 128], bf16, isOutput=False)
    out = nc.declare_dram_parameter("out", [T, C], bf16, isOutput=True)
    # DRAM bounce rows for the partition-broadcast of softmax reciprocals.
    rscr = nc.dram_tensor("rscr", [128, 512], bf16)

    with tile.TileContext(nc) as tc:
        with (
            tc.tile_pool(name="res", bufs=1) as res,
            tc.tile_pool(name="wa", bufs=4) as wa_pool,
            tc.tile_pool(name="wp", bufs=2) as wp_pool,
            tc.tile_pool(name="pt", bufs=6) as pt_pool,
            tc.tile_pool(name="yab", bufs=8) as yab_pool,
            tc.tile_pool(name="small", bufs=4) as small_pool,
            tc.tile_pool(name="qp", bufs=1, space="PSUM") as qp_pool,
            tc.tile_pool(name="op", bufs=1, space="PSUM") as op_pool,
            tc.tile_pool(name="ypl", bufs=2, space="PSUM") as ypl,
            tc.tile_pool(name="ptmp", bufs=2, space="PSUM") as ptmp,
        ):
            xtb = res.tile([128, NCH, T], bf16, tag="xtb", name="xtb")
            xt = [xtb[:, i, :] for i in range(NCH)]
            kt = [res.tile([128, T], bf16, tag=f"kt{i}", name=f"kt{i}") for i in range(NCH)]
            # [NH, 65] V_aug per head + 64 pad cols so every head h has a full
            # 128-wide stationary slice at offset h*65 (FWL-eligible LDW)
            vt = [res.tile([128, NH * (HD + 1) + 64], bf16, tag=f"vt{i}", name=f"vt{i}") for i in range(NT)]
            # double-buffered across query-set parity so q-proj(g+1) overlaps
            # attention(g), and out-proj(g) overlaps attention(g+1)
            qts = [
                [res.tile([128, T], bf16, tag=f"qt{p}_{i}", name=f"qt{p}_{i}") for i in range(NCH)]
                for p in range(2)
            ]
            yts = [
                [res.tile([128, T], bf16, tag=f"yt{p}_{i}", name=f"yt{p}_{i}") for i in range(NCH)]
                for p in range(2)
            ]
            osb = [res.tile([128, C], bf16, tag=f"osb{i}", name=f"osb{i}") for i in range(NT)]

            ident = res.tile([128, 128], bf16, tag="ident", name="ident")
            cmask = res.tile([128, 128], bf16, tag="cmask", name="cmask")
            vtv = [
                vt[tt][:, 0 : NH * (HD + 1)].rearrange("p (a b) -> p a b", b=HD + 1)
                for tt in range(NT)
            ]
            # x tiles first: they gate the first projection matmuls (the Sync
            # DMA queue is serial, so emission order = critical-path order)
            nc.sync.dma_start(out=xtb, in_=xT.rearrange("(a p) t -> p a t", p=128))
            # ones column + pad via on-chip memset instead of strided DMAs
            # (the DMA version cost ~1.4us apiece at the head of the queue)
            for tt in range(NT):
                nc.gpsimd.memset(vt[tt][:, NH * (HD + 1) :], 0.0)
                nc.gpsimd.memset(vtv[tt][:, :, HD : HD + 1], 1.0)

            def project_T(dst, fbase, tag, pool, ptag, fgs=(0, 1)):
                """dst[i][f_local, t] = (x @ W_attn.T).T rows fbase..fbase+1024."""
                for fg in fgs:  # 512-wide feature groups
                    f0 = fbase + fg * 512
                    w = wa_pool.tile([128, NCH, 512], bf16, tag="wa", name=f"wa_{tag}_{fg}")
                    nc.sync.dma_start(
                        out=w,
                        in_=waT[:, f0 : f0 + 512].rearrange("(a p) c -> p a c", p=128),
                    )
                    for tc2 in range(2):
                        for ftl in range(4):
                            ps = pool.tile(
                                [128, 512], f32, tag=ptag,
                                name=f"ps_{tag}_{fg}_{tc2}_{ftl}",
                            )
                            for ct in range(NCH):
                                nc.tensor.matmul(
                                    ps,
                                    w[:, ct, ftl * 128 : (ftl + 1) * 128],
                                    xt[ct][:, tc2 * 512 : (tc2 + 1) * 512],
                                    start=(ct == 0),
                                    stop=(ct == NCH - 1),
                                )
                            fti = fg * 4 + ftl
                            nc.vector.tensor_copy(
                                dst[fti][:, tc2 * 512 : (tc2 + 1) * 512], ps
                            )

            # fg0 halves of K and Q0 first, then V's weights, then the fg1
            # halves: attention(0) j=0 needs kt[0]/qt[0]/vt[*] (fg0+V) at
            # ~17us, while the fg1 halves are only needed from j==4 (~45us).
            project_T(kt, KOFF, "k", qp_pool, "psproj", (0,))
            # q-proj of set 0 on the op pool so it overlaps K/V-proj and lets
            # attention(0) start as early as possible
            project_T(qts[0], 0, "q0", op_pool, "psop", (0,))
            # mask constants aren't needed until the first diagonal block
            # (~40us in) — keep them off the head of the DMA queue
            nc.sync.dma_start(out=ident, in_=identD[:, :])
            nc.sync.dma_start(out=cmask, in_=cmaskD[:, :])

            # V in [token, feature] layout, features interleaved with a ones
            # column every 64 (each head's stationary V_aug slice is [128, 65]).
            wv = []
            for fg in range(2):
                f0 = VOFF + fg * 512
                w = wa_pool.tile([128, NCH, 512], bf16, tag="wa", name=f"wav_{fg}")
                nc.sync.dma_start(
                    out=w,
                    in_=waT[:, f0 : f0 + 512].rearrange("(a p) c -> p a c", p=128),
                )
                wv.append(w)
            # tt-major so each vt[tt] completes early: attention(0)'s first PV
            # matmuls need vt[0..3] long before the full V projection finishes
            for tt in range(NT):
                for fg in range(2):
                    w = wv[fg]
                    ps = op_pool.tile([128, 512], f32, tag="psop", name=f"psv_{fg}_{tt}")
                    for ct in range(NCH):
                        nc.tensor.matmul(
                            ps,
                            xt[ct][:, tt * 128 : (tt + 1) * 128],
                            w[:, ct, :],
                            start=(ct == 0),
                            stop=(ct == NCH - 1),
                        )
                    nc.vector.tensor_copy(
                        vtv[tt][:, fg * 8 : (fg + 1) * 8, 0:HD],
                        ps.rearrange("p (a b) -> p a b", b=HD),
                    )
            project_T(kt, KOFF, "kb", qp_pool, "psproj", (1,))
            project_T(qts[0], 0, "q0b", op_pool, "psop", (1,))

            wp_cache = {}

            def load_wp(pg, cc):
                """Dispatch the W_proj half-load one head-pair ahead of its
                chains, from the idle gpsimd queue so the 2.9us transfer never
                queues in front of the latency-critical normalize DMAs."""
                wp = wp_pool.tile([128, NCH, 512], bf16, tag="wp", name=f"wp{pg}_{cc}")
                nc.gpsimd.dma_start(
                    out=wp,
                    in_=wpT[
                        pg * C : (pg + 1) * C, cc * 512 : (cc + 1) * 512
                    ].rearrange("(a p) c -> p a c", p=128),
                )
                wp_cache[(pg, cc)] = wp

            def emit_outproj(pg, cc, tts=None):
                """One cc half of out-projection for set pg, into osb.

                Emitted inside attention(pg+1)'s j loop so the scheduler
                keeps it as late PE filler there; set 3 is emitted after its
                own attention and alternates over all four idle PSUM pools.
                """
                pyt = yts[pg % 2]
                wp = wp3[cc] if pg == NQS - 1 else wp_cache.get((pg, cc))
                for tt in (range(NT) if tts is None else tts):
                    if pg == NQS - 1:
                        pool2, tg = [
                            (op_pool, "psop"), (ypl, "ypl"),
                            (qp_pool, "psproj"), (ypl, "ypl"),
                        ][tt % 4]
                    else:
                        pool2, tg = op_pool, "psop"
                    ps = pool2.tile(
                        [128, 512], f32, tag=tg, name=f"psp{pg}_{cc}_{tt}"
                    )
                    for ftl in range(NCH):
                        nc.tensor.matmul(
                            ps,
                            pyt[ftl][:, tt * 128 : (tt + 1) * 128],
                            wp[:, ftl, :],
                            start=(ftl == 0),
                            stop=(ftl == NCH - 1),
                        )
                    dst = osb[tt][:, cc * 512 : (cc + 1) * 512]
                    if pg == 0:
                        nc.vector.tensor_copy(dst, ps)
                    else:
                        nc.vector.tensor_add(dst, dst, ps)
                    if pg == NQS - 1:
                        # stream each finished half out immediately so the
                        # store drain overlaps the remaining chains
                        nc.sync.dma_start(
                            out=out[tt * 128 : (tt + 1) * 128, cc * 512 : (cc + 1) * 512],
                            in_=dst,
                        )

            for g in range(NQS):
                qt = qts[g % 2]
                yt = yts[g % 2]
                if g > 0:
                    # for the last set, hold back the fg1 half: emitted inside
                    # the j loop below so the scheduler keeps it as PE filler
                    # for attention(3) (which has no q-proj of its own)
                    fgs = (0,) if g == NQS - 1 else (0, 1)
                    project_T(qt, g * C, f"q{g}", qp_pool, "psproj", fgs)
                if g == NQS - 1:
                    # both cc halves of W_proj for the last set, via the wp
                    # pool (free once set-2's chains consumed their weights),
                    # so the per-j out-projection of set 3 never waits on DMA
                    wp3 = []
                    for cc in range(2):
                        w3 = wa_pool.tile(
                            [128, NCH, 512], bf16, tag="wa", name=f"wp3_{cc}"
                        )
                        nc.gpsimd.dma_start(
                            out=w3,
                            in_=wpT[
                                3 * C : 4 * C, cc * 512 : (cc + 1) * 512
                            ].rearrange("(a p) c -> p a c", p=128),
                        )
                        wp3.append(w3)

                for j in range(NH // 2):  # head pairs (2j, 2j+1)
                    if g == NQS - 1 and j == 2:
                        project_T(qt, g * C, f"q{g}b", qp_pool, "psproj", (1,))
                    if g > 0 and j == 0:
                        load_wp(g - 1, 0)
                    if g > 0 and j == 1:
                        emit_outproj(g - 1, 0)
                    if g > 0 and j == 3:
                        load_wp(g - 1, 1)
                    # in the last window, split the cc1 chains across j==6
                    # and j==7 so they bridge BOTH observed idle regions (mid
                    # j6-7 and the final-normalize-to-last-projection gap);
                    # they read yt2, cold for a full window, so the late
                    # placement cannot block the in-order PE queue
                    if g > 0:
                        if g == NQS - 1:
                            if j == 6:
                                emit_outproj(g - 1, 1, range(0, NT // 2))
                            if j == 7:
                                emit_outproj(g - 1, 1, range(NT // 2, NT))
                        elif j == 5:
                            emit_outproj(g - 1, 1)
                    for qc in range(2):  # 512-wide query chunks
                        nkt = 4 * qc + 4
                        yps = [
                            ypl.tile([128, 512], f32, tag="ypl", name=f"yp{g}_{j}_{qc}_{hh}")
                            for hh in range(2)
                        ]
                        # software-pipelined: QK(k2)+exp(k2) emitted one step
                        # ahead of PV(k2-1) so the PE never waits on ACT
                        pts = [None] * nkt
                        geom = []
                        for k2 in range(nkt):
                            qlo = max(qc * 512, k2 * 128)
                            wdt = qc * 512 + 512 - qlo
                            geom.append((qlo, wdt))
                        for k2 in range(nkt + 1):
                            if k2 < nkt:
                                qlo, wdt = geom[k2]
                                diag = k2 * 128 >= qc * 512
                                sp = ptmp.tile(
                                    [128, 1024], f32, tag="ptmp",
                                    name=f"sp{g}_{j}_{qc}_{k2}",
                                )
                                for hh in range(2):
                                    nc.tensor.matmul(
                                        sp[:, hh * 512 : hh * 512 + wdt],
                                        kt[j][hh * 64 : hh * 64 + 64, k2 * 128 : (k2 + 1) * 128],
                                        qt[j][hh * 64 : hh * 64 + 64, qlo : qlo + wdt],
                                        start=True,
                                        stop=not diag,
                                    )
                                if diag:
                                    # additive causal mask (0 / -1e30) on the
                                    # 128x128 diagonal blocks, applied on the
                                    # PE so the exp/PV deps stay single-engine
                                    for hh in range(2):
                                        nc.tensor.matmul(
                                            sp[:, hh * 512 : hh * 512 + 128],
                                            cmask,
                                            ident,
                                            start=False,
                                            stop=True,
                                            skip_group_check=True,
                                        )
                                pt = pt_pool.tile(
                                    [128, 2, 512], bf16, tag="pt",
                                    name=f"pt{g}_{j}_{qc}_{k2}",
                                )
                                nc.scalar.activation(
                                    pt[:, :, 0:wdt],
                                    sp.rearrange("p (a c) -> p a c", a=2)[:, :, 0:wdt],
                                    EXP,
                                    bias=0.0,
                                    scale=SCALE,
                                )
                                pts[k2] = pt
                            if k2 > 0:
                                qlo, wdt = geom[k2 - 1]
                                off = qlo - qc * 512
                                for hh in range(2):
                                    h = 2 * j + hh
                                    # 128-wide stationary slice (V_aug of head
                                    # h + spillover) -> FWL background load;
                                    # out rows 65-127 are unused garbage
                                    nc.tensor.matmul(
                                        yps[hh][:, off : off + wdt],
                                        vt[k2 - 1][:, h * (HD + 1) : h * (HD + 1) + 128],
                                        pts[k2 - 1][:, hh, 0:wdt],
                                        start=(k2 - 1 == 0),
                                        stop=(k2 - 1 == nkt - 1),
                                    )
                        # phase 1: BOTH casts first - each cast is the sole
                        # reader of its yps PSUM bank, and the bank release
                        # gates the next (j,qc) block's PV accumulation. The
                        # DMA-bounce-dependent ops go afterwards so a DRAM
                        # round trip never sits ahead of a cast in the queue.
                        yabs = []
                        for hh in range(2):
                            yab = yab_pool.tile(
                                [65, 512], bf16, tag="yab", name=f"yab{g}_{j}_{qc}_{hh}"
                            )
                            nc.vector.tensor_copy(yab, yps[hh][0:65, :])
                            yabs.append(yab)
                        # phase 2: spread the single-partition denominator row
                        # to [64, 8] lanes (SBUF->SBUF DMA), reciprocal, then
                        # DRAM hop for the partition-broadcast back
                        # DRAM bounce for the partition-broadcast: SBUF APs
                        # reject zero partition stride (verified), so the
                        # reciprocal rows must round-trip through DRAM
                        ci0 = ((g * 8 + j) * 2 + qc) * 2
                        # both den scatters dispatched before either recip so
                        # their DMA latencies overlap; the recips are the only
                        # DMA-waiting ops left ahead of the next block's casts
                        # in the vector queue
                        den2 = small_pool.tile(
                            [64, 16], bf16, tag="den64", name=f"den{g}_{j}_{qc}"
                        )
                        for hh in range(2):
                            nc.sync.dma_start(
                                out=den2[:, hh * 8 : (hh + 1) * 8],
                                in_=yabs[hh][64:65, :],
                            )
                        # ONE reciprocal for both heads (one fewer DMA-waiting
                        # op ahead of the next block's casts in the queue)
                        rec2 = small_pool.tile(
                            [64, 16], bf16, tag="rec64", name=f"rec{g}_{j}_{qc}"
                        )
                        with nc.allow_low_precision("bf16 softmax recip ok at 2e-2 tol"):
                            nc.vector.reciprocal(out=rec2, in_=den2)
                        for hh in range(2):
                            drow = rscr[ci0 + hh : ci0 + hh + 1, :]
                            nc.sync.dma_start(
                                out=drow.rearrange("a (b c) -> (a b) c", b=64),
                                in_=rec2[:, hh * 8 : (hh + 1) * 8],
                            )
                        # ONE partition-broadcast DMA for both heads' rows
                        # (halves the dispatch count on the sync queue)
                        drow2 = rscr[ci0 : ci0 + 2, :]
                        bc2 = small_pool.tile(
                            [64, 2, 512], bf16, tag="bcst", name=f"bcst{g}_{j}_{qc}"
                        )
                        nc.sync.dma_start(
                            out=bc2,
                            in_=bass.AP(
                                tensor=drow2.tensor,
                                offset=drow2.offset,
                                ap=[[0, 64]] + drow2.ap,
                            ),
                        )
                        # phase 3: normalize into yt (feeds the out-projection)
                        # on gpsimd: all-SBUF operands, ~10us of slack, and it
                        # keeps the bc2-waiting ops out of the vector queue.
                        # Exception: the LAST block sits on the tail critical
                        # path (final chains' ftl=7 wait on it) and vector is
                        # idle there - avoid gpsimd's ~1us Q7 launch latency.
                        meng = (
                            nc.vector if (g == NQS - 1 and j == NH // 2 - 1)
                            else nc.gpsimd
                        )
                        for hh in range(2):
                            meng.tensor_mul(
                                yt[j][hh * 64 : hh * 64 + 64, qc * 512 : qc * 512 + 512],
                                yabs[hh][0:64, :],
                                bc2[:, hh, :],
                            )

                if g == NQS - 1:
                    for cc in range(2):
                        emit_outproj(g, cc)



    nc.compile()
    return nc


def kernel(x, W_attn, W_proj, _trace=False):
    if "nc" not in _CACHE:
        _CACHE["nc"] = _build()
    nc = _CACHE["nc"]

    xT = np.ascontiguousarray(np.transpose(np.asarray(x, np.float32), (0, 2, 1))).astype(BF16)
    waT = np.ascontiguousarray(np.asarray(W_attn, np.float32).T).astype(BF16)
    wpT = np.ascontiguousarray(np.asarray(W_proj, np.float32).T).astype(BF16)
    ii = np.arange(128)
    ident = np.eye(128, dtype=np.float32).astype(BF16)
    # lhsT for the mask matmul: out[k,q] = cmaskT[q,k] = 0 if q>=k else -1e30
    cmaskT = (
        np.where(ii[:, None] >= ii[None, :], 0.0, -1e30)
        .astype(np.float32)
        .astype(BF16)
    )

    in_maps = [
        {"xT": xT[b], "waT": waT, "wpT": wpT, "ident": ident, "cmaskT": cmaskT}
        for b in range(B)
    ]
    res = run_bass_kernel_spmd(nc, in_maps, core_ids=list(range(B)), trace=_trace)
    LAST["exec_time_ns"] = res.exec_time_ns
    LAST["mean_exec_time_ns"] = res.mean_exec_time_ns
    LAST["results"] = res
    return np.stack([res.results[b]["out"] for b in range(B)]).astype(np.float32)

